# revision 34
# baseline (speedup 1.0000x reference)
"""Trainium2 Bass kernel for nn_CrossAttention (B=2, C=512, N=M=2048, H=8).

Sharding: batch*heads = 16 (b,h) pairs across 8 cores, 2 heads per core.
Cores 0-3 handle batch 0 (heads in pairs), cores 4-7 batch 1.

Per-core pipeline (bf16 compute, fp32 PSUM accumulation):
  kT[d,m] = Wk_cols.T @ y_b          (2 heads packed on partitions)
  qT[d,n] = (Wq_cols * SCALE).T @ x_b
  v2[m, 1+d | 1+d] = y_blk.T @ Wv'   (direct [m,d] layout, ones cols preset;
                                      Wv' has the depthwise conv folded in)
  S^T[m,n] = kT_h.T-slices @ qT_h    (row-packed K=64 pairs per head)
  P = exp(S^T) -> bf16               (ScalarE streaming [128,1024] blocks)
  acc[n, 1+d] += P_blk.T @ v2[m]     (flipped attnout: P is the stationary,
                                      65-wide free -> 65 cyc/matmul; col 0 of
                                      each group accumulates the denominator;
                                      one start/stop per PSUM bank since the
                                      start bit zeroes the whole 2KB bank)
  nrm[n, d] = acc * recip(den)       (DVE per-partition scalar multiply)
  attnT[c, n] = PE-transpose(nrm)    (bf16)
  outT_partial[cout, n] = Wp_rows.T @ attnT   (bf16 partials to HBM)

Host folds (1+lw) into Wv, bias' = bp + lb @ Wp (exact: softmax rows sum
to 1), sums the 4 per-batch partials, adds bias'.
"""

import os
import sys
import numpy as np
from contextlib import ExitStack

for _p in ("/root/.axon_site", "/root/.axon_site/_ro/trn_rl_repo",
           "/root/.axon_site/_ro/pypackages", "/opt/trn_rl_repo"):
    if os.path.isdir(_p) and _p not in sys.path:
        sys.path.append(_p)

B, C, N, M, H = 2, 512, 2048, 2048, 8
HD = C // H
SCALE = HD ** -0.5
NCORES = 8

_NC = None
LAST_RUN = None


def _build_program(reps=1):
    from concourse import bacc
    import concourse.tile as tile
    import concourse.mybir as mybir
    from concourse.masks import make_identity

    F32 = mybir.dt.float32
    BF16 = mybir.dt.bfloat16
    EXP = mybir.ActivationFunctionType.Exp
    COPY = mybir.ActivationFunctionType.Copy
    MULT = mybir.AluOpType.mult

    nc = bacc.Bacc("TRN2", target_bir_lowering=False, debug=False,
                   num_devices=NCORES)

    xr = nc.dram_tensor("xr", [C, N], BF16, kind="ExternalInput").ap()
    yr = nc.dram_tensor("yr", [C, M], BF16, kind="ExternalInput").ap()
    # wall = [Wk' | Wq' | Wv'] concatenated so one DMA loads all three
    wall_d = nc.dram_tensor("wall", [C, 384], BF16, kind="ExternalInput").ap()
    wp_d = nc.dram_tensor("wp", [128, C], BF16, kind="ExternalInput").ap()
    outT = nc.dram_tensor("outT", [C, N], BF16, kind="ExternalOutput").ap()

    xr4 = xr.rearrange("(kc p) n -> p kc n", p=128)
    yr4 = yr.rearrange("(kc p) n -> p kc n", p=128)
    outT4 = outT.rearrange("(cc p) n -> p cc n", p=128)

    with tile.TileContext(nc) as tc, ExitStack() as ctx:
        sb = ctx.enter_context(tc.tile_pool(name="sb", bufs=1))
        ppool = ctx.enter_context(tc.tile_pool(name="ppool", bufs=5))
        npool = ctx.enter_context(tc.tile_pool(name="npool", bufs=2))
        spool = ctx.enter_context(tc.tile_pool(name="spool", bufs=2))
        # PSUM budget (8 banks): psA ring 3x[128,1024]f32 = 6 banks (scores,
        # proj/v2 staging, transposes, outproj transients); psB 2x1 bank
        # (attnout accumulators; the tail reuses them for outproj).
        psA = ctx.enter_context(tc.tile_pool(name="psA", bufs=3, space="PSUM"))
        psB = ctx.enter_context(tc.tile_pool(name="psB", bufs=2, space="PSUM"))

        # ---- PE warm-up with no DMA dependency: DVE-zeroed operand ----
        zwarm = sb.tile([128, 128], F32, tag="zwarm")
        nc.vector.memset(zwarm, 0.0)
        # warm the exp table while DMAs stream
        warm = sb.tile([1, 32], F32, tag="warm")
        nc.scalar.activation(warm, zwarm[0:1, 0:32], EXP)
        psw = psA.tile([128, 128], F32, tag="blk", name="psw")
        for _ in range(7):
            nc.tensor.matmul(psw, zwarm, zwarm, start=True, stop=True)
        warm2 = sb.tile([128, 128], F32, tag="warm2")
        nc.vector.tensor_copy(warm2, psw)

        # ---- input DMAs, all on the sync-engine HWDGE queue; order is
        # the first-use order so the global DMA serialization helps the
        # prologue rather than hurting it ----
        wall_sb = sb.tile([128, 4, 384], BF16, tag="wall_sb")
        wp_sb = sb.tile([128, C], BF16, tag="wp_sb")
        y_sb = sb.tile([128, 4, M], BF16, tag="y_sb")
        x_sb = sb.tile([128, 4, N], BF16, tag="x_sb")
        wk_sb = wall_sb[:, :, 0:128]
        wq_sb = wall_sb[:, :, 128:256]
        wv_sb = wall_sb[:, :, 256:384]

        def load_j(dst, src, j):
            nc.sync.dma_start(out=dst[:, :, j * 512:(j + 1) * 512],
                              in_=src[:, :, j * 512:(j + 1) * 512])

        def load_half(dst, src, h):
            nc.sync.dma_start(out=dst[:, :, h * 256:(h + 1) * 256],
                              in_=src[:, :, h * 256:(h + 1) * 256])

        nc.sync.dma_start(
            out=wall_sb, in_=wall_d.rearrange("(kc p) m -> p kc m", p=128))
        # first j in 256-halves so projections start at half-arrival
        load_half(y_sb, yr4, 0)
        load_half(y_sb, yr4, 1)
        load_half(x_sb, xr4, 0)
        load_half(x_sb, xr4, 1)
        load_j(y_sb, yr4, 1)
        load_j(x_sb, xr4, 1)
        nc.sync.dma_start(out=wp_sb, in_=wp_d)
        load_j(y_sb, yr4, 2)
        load_j(x_sb, xr4, 2)
        load_j(y_sb, yr4, 3)
        load_j(x_sb, xr4, 3)

        # identity (bf16, for PE transposes) built on the idle GPSIMD
        identb = sb.tile([128, 128], BF16, tag="identb")
        make_identity(nc, identb)

        # v2[p, mb, col]: col 0 = ones (head a den), 1..64 = head a values,
        # col 65 = ones (head b den), 66..129 = head b values.
        v2 = sb.tile([128, 16, 130], BF16, tag="v2")
        nc.vector.memset(v2[:, :, 0:1], 1.0)
        nc.vector.memset(v2[:, :, 65:66], 1.0)

        kT = sb.tile([128, M], BF16, tag="kT")
        qT = sb.tile([128, N], BF16, tag="qT")

        hold = {}

        def proj_half(dst, w_sb, src, j, half, name):
            if half == 0:
                hold[name] = psA.tile([128, 512], F32, tag="blk", name=name)
            ps = hold[name]
            for kc in (0, 1) if half == 0 else (2, 3):
                nc.tensor.matmul(ps, w_sb[:, kc, :],
                                 src[:, kc, j * 512:(j + 1) * 512],
                                 start=(kc == 0), stop=(kc == 3))
            if half == 1:
                nc.vector.tensor_copy(dst[:, j * 512:(j + 1) * 512], ps)

        def v2_task(mb):
            ps = psA.tile([128, 128], F32, tag="blk", name=f"v2ps{mb}")
            ms = slice(mb * 128, (mb + 1) * 128)
            for kc in range(4):
                nc.tensor.matmul(ps, y_sb[:, kc, ms], wv_sb[:, kc, :],
                                 start=(kc == 0), stop=(kc == 3))
            nc.vector.tensor_copy(v2[:, mb, 1:65], ps[:, 0:64])
            nc.vector.tensor_copy(v2[:, mb, 66:130], ps[:, 64:128])

        def proj256(dst, w_sb, src, h, name):
            # 256-wide projection so work starts at half-DMA arrival
            ps = psA.tile([128, 256], F32, tag="blk", name=name)
            sl = slice(h * 256, (h + 1) * 256)
            for kc in range(4):
                nc.tensor.matmul(ps, w_sb[:, kc, :], src[:, kc, sl],
                                 start=(kc == 0), stop=(kc == 3))
            nc.vector.tensor_copy(dst[:, sl], ps)

        # ---- prologue: only what gates scores(chunk0, m=0..3); the qT
        # copies go ahead of the v2 copies on the DVE queue since the
        # first exp gates on qT while v2 is only needed two steps in ----
        proj256(kT, wk_sb, y_sb, 0, "psk0a")
        proj256(kT, wk_sb, y_sb, 1, "psk0b")
        proj256(qT, wq_sb, x_sb, 0, "psq0a")
        proj256(qT, wq_sb, x_sb, 1, "psq0b")

        # weave tasks: late projections + v2 blocks. Emission order defines
        # data dependencies (a consumer emitted before its producer reads
        # stale SBUF), so each task must be emitted strictly before its
        # first consumer: kT j before scores(0, 4j), v2(mb) before the
        # attnout(mb) emission (mb+2), qT j before scores(j, 0).
        def P_(dst, w, src, j, half, name):
            return lambda: proj_half(dst, w, src, j, half, name)

        fills = {
            (0, 0): [lambda: v2_task(0), lambda: v2_task(1)],
            (0, 1): [lambda: v2_task(2), lambda: v2_task(3)],
            (0, 2): [P_(kT, wk_sb, y_sb, 1, 0, "psk1")],
            (0, 3): [P_(kT, wk_sb, y_sb, 1, 1, "psk1")],
            (0, 4): [lambda: v2_task(4)],
            (0, 5): [lambda: v2_task(5)],
            (0, 6): [P_(kT, wk_sb, y_sb, 2, 0, "psk2"), lambda: v2_task(6)],
            (0, 7): [P_(kT, wk_sb, y_sb, 2, 1, "psk2"), lambda: v2_task(7)],
            (0, 8): [lambda: v2_task(8)],
            (0, 9): [lambda: v2_task(9)],
            (0, 10): [P_(kT, wk_sb, y_sb, 3, 0, "psk3"), lambda: v2_task(10)],
            (0, 11): [P_(kT, wk_sb, y_sb, 3, 1, "psk3"), lambda: v2_task(11)],
            (0, 12): [lambda: v2_task(12)],
            (0, 13): [P_(qT, wq_sb, x_sb, 1, 0, "psq1"), lambda: v2_task(13)],
            (0, 14): [P_(qT, wq_sb, x_sb, 1, 1, "psq1"), lambda: v2_task(14)],
            (0, 15): [lambda: v2_task(15)],
            (1, 1): [P_(qT, wq_sb, x_sb, 2, 0, "psq2")],
            (1, 2): [P_(qT, wq_sb, x_sb, 2, 1, "psq2")],
            (2, 1): [P_(qT, wq_sb, x_sb, 3, 0, "psq3")],
            (2, 2): [P_(qT, wq_sb, x_sb, 3, 1, "psq3")],
        }

        # ---- main attention loop ----
        pending = []         # (P, acc_a, acc_b, m) awaiting attnout
        post = []            # deferred post-processing closures

        def emit_attnout(P, acc_a, acc_b, m):
            # PSUM zero-region = one full 2KB bank: exactly one start
            # (m=0,s=0) and one stop (m=15,s=3) per accumulator tile.
            for s in range(4):
                if isinstance(P, tuple):  # split first m-step: [a256|b256]x2
                    Pt = P[1]
                    h, i = s // 2, s % 2
                    pa = Pt[:, h * 512 + i * 128:h * 512 + (i + 1) * 128]
                    pb = Pt[:, h * 512 + 256 + i * 128:
                            h * 512 + 256 + (i + 1) * 128]
                else:
                    pa = P[:, s * 128:(s + 1) * 128]
                    pb = P[:, 512 + s * 128:512 + (s + 1) * 128]
                nc.tensor.matmul(acc_a[:, s, :], pa, v2[:, m, 0:65],
                                 start=(m == 0 and s == 0),
                                 stop=(m == 15 and s == 3))
                nc.tensor.matmul(acc_b[:, s, :], pb, v2[:, m, 65:130],
                                 start=(m == 0 and s == 0),
                                 stop=(m == 15 and s == 3))

        def make_post(c, acc_a, acc_b):
            st = {}

            def grab(aps, dst_tag, out_name):
                # one fast PSUM->SBUF copy releases the accumulator bank
                t = spool.tile([128, 4, 65], F32, tag=dst_tag, name=out_name)
                nc.vector.tensor_copy(t, aps)
                return t

            def grab_a():
                st["ca"] = grab(acc_a, "ca", f"ca{c}")
                st["cb"] = grab(acc_b, "cb", f"cb{c}")

            def recips():
                st["ra"] = spool.tile([128, 4], F32, tag="ra", name=f"ra{c}")
                st["rb"] = spool.tile([128, 4], F32, tag="rb", name=f"rb{c}")
                nc.vector.reciprocal(st["ra"], st["ca"][:, :, 0:1])
                nc.vector.reciprocal(st["rb"], st["cb"][:, :, 0:1])
                st["attnT"] = npool.tile([128, 512], BF16, tag="attnT",
                                         name=f"attnT{c}")

            def sub(s):
                trin = spool.tile([128, 128], BF16, tag=f"trin{s % 2}",
                                  name=f"trin{c}_{s}")
                nc.vector.tensor_scalar(trin[:, 0:64], st["ca"][:, s, 1:65],
                                        st["ra"][:, s:s + 1], None, op0=MULT)
                nc.vector.tensor_scalar(trin[:, 64:128], st["cb"][:, s, 1:65],
                                        st["rb"][:, s:s + 1], None, op0=MULT)
                tp = psA.tile([128, 128], BF16, tag="blk", name=f"tp{c}_{s}")
                nc.tensor.transpose(tp, trin, identb)
                nc.vector.tensor_copy(
                    st["attnT"][:, s * 128:(s + 1) * 128], tp)

            def outproj(cc):
                po = psA.tile([128, 512], F32, tag="blk", name=f"po{c}_{cc}")
                nc.tensor.matmul(po, wp_sb[:, cc * 128:(cc + 1) * 128],
                                 st["attnT"], start=True, stop=True)
                if cc == 0:
                    st["so"] = npool.tile([128, 4, 512], BF16, tag="so",
                                          name=f"so{c}")
                nc.vector.tensor_copy(st["so"][:, cc, :], po)
                if cc == 3:
                    nc.sync.dma_start(
                        out=outT4[:, :, c * 512:(c + 1) * 512],
                        in_=st["so"])

            return ([[grab_a], [recips]]
                    + [[lambda s=s: sub(s)] for s in range(4)]
                    + [[lambda cc=cc: outproj(cc)] for cc in range(4)])

        for c in range(4):
            ns = slice(c * 512, (c + 1) * 512)
            acc_a = psB.tile([128, 4, 65], F32, tag="acc", name=f"acca{c}")
            acc_b = psB.tile([128, 4, 65], F32, tag="acc", name=f"accb{c}")
            for m in range(16):
                ms = slice(m * 128, (m + 1) * 128)
                if c == 0 and m == 0:
                    # first m-step split into two per-head-pair [128,512]
                    # pieces gated on the x j0 DMA *halves*, so the exp
                    # stream starts ~1.5us earlier. Layout per half h:
                    # P[:, h*512:(h+1)*512] = [a n256 | b n256].
                    P = ppool.tile([128, 1024], BF16, tag="p", name="p0_0")
                    for h in range(2):
                        nh = slice(h * 256, (h + 1) * 256)
                        # tile_position'd matmuls may not share a PSUM bank:
                        # head pieces go to separate banks, exp reads the
                        # pair through a strided AP.
                        blk = psA.tile([128, 2, 512], F32, tag="blk",
                                       name=f"blk0_0{h}")
                        nc.tensor.matmul(blk[:, 0, 0:256], kT[0:64, ms],
                                         qT[0:64, nh], start=True, stop=True,
                                         tile_position=(0, 0))
                        nc.tensor.matmul(blk[:, 1, 0:256], kT[64:128, ms],
                                         qT[64:128, nh], start=True,
                                         stop=True, tile_position=(64, 0))
                        nc.scalar.activation(P[:, h * 512:(h + 1) * 512],
                                             blk[:, :, 0:256], EXP)
                    P = ("split", P)
                else:
                    blk = psA.tile([128, 1024], F32, tag="blk",
                                   name=f"blk{c}_{m}")
                    nc.tensor.matmul(blk[:, 0:512], kT[0:64, ms],
                                     qT[0:64, ns],
                                     start=True, stop=True,
                                     tile_position=(0, 0))
                    nc.tensor.matmul(blk[:, 512:1024], kT[64:128, ms],
                                     qT[64:128, ns],
                                     start=True, stop=True,
                                     tile_position=(64, 0))
                    P = ppool.tile([128, 1024], BF16, tag="p",
                                   name=f"p{c}_{m}")
                    nc.scalar.activation(P, blk, EXP)
                if post and m >= 2:
                    for task in post.pop(0):
                        task()
                for task in fills.pop((c, m), ()):
                    task()
                pending.append((P, acc_a, acc_b, m))
                # lag 2 normally; a new chunk's first attnout (which waits
                # for the previous accumulator bank to be copied out by
                # grab_a) is held until m=3 so it never stalls the in-order
                # PE queue ahead of the score stream; the last two steps of
                # the last chunk defer entirely so the final exps are not
                # delayed behind attnout matmuls.
                while len(pending) > 2 and not (
                        c > 0 and m < 3 and pending[0][3] == 0) and not (
                        c == 3 and m >= 14):
                    emit_attnout(*pending.pop(0))
            post = make_post(c, acc_a, acc_b)

        # ---- drain + tail (chunk 3 post-processing, pipelined) ----
        # Read the accumulators straight from PSUM (no ring pressure at
        # the end), split normalize/copy work across DVE and the now-idle
        # ScalarE (Copy shares the exp activation table, no reload), and
        # run the output projection per n-sub-block so copies and DMAs
        # start as early as possible. Two po tiles reuse the accumulator
        # banks freed at the start of the tail.
        while pending:
            emit_attnout(*pending.pop(0))
        c = 3
        ra = spool.tile([128, 4], F32, tag="ra", name="ra3")
        rb = spool.tile([128, 4], F32, tag="rb", name="rb3")
        nc.vector.reciprocal(ra, acc_a[:, :, 0:1])
        nc.vector.reciprocal(rb, acc_b[:, :, 0:1])
        attnT3 = npool.tile([128, 512], BF16, tag="attnT", name="attnT3")
        trins = [spool.tile([128, 128], BF16, tag="ttr", name=f"trin3_{s}",
                            bufs=4) for s in range(4)]
        # per-engine streams with no cross-engine ping-pong: DVE runs all
        # normalizes then the transpose copies; PE runs transposes then
        # the per-sub output projections; ScalarE+DVE split the output
        # copies; two DMAs so the first half ships early.
        po = [psB.tile([128, 512], F32, tag="acc", name="po3_0"),
              psB.tile([128, 512], F32, tag="acc", name="po3_1"),
              psA.tile([128, 512], F32, tag="blk", name="po3_2"),
              psA.tile([128, 512], F32, tag="blk", name="po3_3")]
        for s in range(4):
            nc.vector.tensor_scalar(trins[s][:, 0:64], acc_a[:, s, 1:65],
                                    ra[:, s:s + 1], None, op0=MULT)
            nc.vector.tensor_scalar(trins[s][:, 64:128], acc_b[:, s, 1:65],
                                    rb[:, s:s + 1], None, op0=MULT)
        tps = []
        for s in range(4):
            tp = psA.tile([128, 128], BF16, tag="blk", name=f"tp3_{s}")
            nc.tensor.transpose(tp, trins[s], identb)
            tps.append(tp)
        for s in range(4):
            nc.vector.tensor_copy(attnT3[:, s * 128:(s + 1) * 128], tps[s])
        for s in range(4):
            for cc in range(4):
                nc.tensor.matmul(po[cc][:, s * 128:(s + 1) * 128],
                                 wp_sb[:, cc * 128:(cc + 1) * 128],
                                 attnT3[:, s * 128:(s + 1) * 128],
                                 start=(s == 0), stop=(s == 3))
        so3 = npool.tile([128, 4, 512], BF16, tag="so", name="so3")
        nc.vector.tensor_copy(so3[:, 0, :], po[0])
        nc.scalar.copy(so3[:, 1, :], po[1])
        nc.sync.dma_start(out=outT4[:, 0:2, c * 512:(c + 1) * 512],
                          in_=so3[:, 0:2, :])
        nc.vector.tensor_copy(so3[:, 2, :], po[2])
        nc.scalar.copy(so3[:, 3, :], po[3])
        nc.sync.dma_start(out=outT4[:, 2:4, c * 512:(c + 1) * 512],
                          in_=so3[:, 2:4, :])

    nc.compile()
    return nc


def _get_program():
    global _NC
    if _NC is None:
        _NC = _build_program()
    return _NC


def make_in_maps(inputs):
    import ml_dtypes
    bf16 = ml_dtypes.bfloat16

    x = np.asarray(inputs["x"], np.float32)
    y = np.asarray(inputs["y"], np.float32)
    Wq = np.asarray(inputs["Wq"], np.float32)
    Wkv = np.asarray(inputs["Wkv"], np.float32)
    lw = np.asarray(inputs["lw"], np.float32)
    Wp = np.asarray(inputs["Wp"], np.float32)

    d = np.arange(HD)
    xb = [np.ascontiguousarray(x[b]).astype(bf16) for b in range(B)]
    yb = [np.ascontiguousarray(y[b]).astype(bf16) for b in range(B)]
    in_maps = []
    for core in range(NCORES):
        b = core // 4
        h0 = (core % 4) * 2
        ch = np.concatenate([h * HD + d for h in (h0, h0 + 1)])  # channels
        colsK = np.concatenate([h * 2 * HD + 2 * d for h in (h0, h0 + 1)])
        wq_c = Wq[:, ch] * np.float32(SCALE)
        wk_c = Wkv[:, colsK]
        wv_c = Wkv[:, colsK + 1] * (1.0 + lw[ch])[None, :]
        wp_c = Wp[ch, :]
        wall = np.concatenate([wk_c, wq_c, wv_c], axis=1)  # [C, 384]
        in_maps.append({
            "xr": xb[b],
            "yr": yb[b],
            "wall": np.ascontiguousarray(wall).astype(bf16),
            "wp": np.ascontiguousarray(wp_c).astype(bf16),
        })
    return in_maps


def assemble_output(results, inputs):
    lb = np.asarray(inputs["lb"], np.float32)
    Wp = np.asarray(inputs["Wp"], np.float32)
    bp = np.asarray(inputs["bp"], np.float32)
    bias = (bp + lb @ Wp).astype(np.float32)
    parts = [np.asarray(results[i]["outT"], dtype=np.float32)
             for i in range(NCORES)]
    out = np.stack([
        parts[0] + parts[1] + parts[2] + parts[3],
        parts[4] + parts[5] + parts[6] + parts[7],
    ])
    out += bias[None, :, None]
    return out.astype(np.float32)


def kernel(x, y, Wq, Wkv, lw, lb, Wp, bp):
    global LAST_RUN
    from concourse.bass_utils import run_bass_kernel_spmd

    inputs = dict(x=x, y=y, Wq=Wq, Wkv=Wkv, lw=lw, lb=lb, Wp=Wp, bp=bp)
    nc = _get_program()
    in_maps = make_in_maps(inputs)
    LAST_RUN = run_bass_kernel_spmd(nc, in_maps, list(range(NCORES)))
    return assemble_output(LAST_RUN.results, inputs)
